# revision 14
# baseline (speedup 1.0000x reference)
# Trainium2 Bass kernel for the ContextBlock problem.
#
# Reference computation (per sample b):
#   xc    = concat(x0..x3)            [C=1024, HW=4096]
#   attn  = softmax(wm @ xc)          [HW]
#   ctx   = xc @ attn                 [C]
#   mul   = residual-gated MLP stack (sigmoid branch)   [C]
#   add   = residual-gated MLP stack (linear branch)    [C]
#   out   = sum_l (x_l * mul_l + add_l)                 [CL=256, HW]
#
# Distribution: data-parallel over batch, one sample per NeuronCore (B=8).
# No collectives required.
#
# Per-core dataflow (v2):
#   All weights prefetched up front (few, large DMAs) so gate layers never
#   wait on HBM. pass1 logits ride the x DMA stream. Softmax exp runs on
#   the Act engine with table switches hidden by dummy ops (sigmoid is
#   computed via tanh, which shares the exp table set). pass2 (context)
#   is column-split across DVE and GpSimd; the r0 gate W1 matmuls overlap
#   it pair-by-pair. Gate branches (mul/add) are stacked so each repeat is
#   one W1 stream + one shared LN pipeline + one W2 stream. W1 uses fp8
#   DoubleRow (2 k-slabs per matmul); weights and v0 are pre-scaled x16
#   with the LN eps scaled to compensate exactly. Output is staged bf16.

import numpy as np
import ml_dtypes
from contextlib import ExitStack

import concourse.bass as bass
import concourse.bacc as bacc
import concourse.mybir as mybir
import concourse.tile as tile

BF = mybir.dt.bfloat16
F32 = mybir.dt.float32
F8 = mybir.dt.float8e4
AF = mybir.ActivationFunctionType
ALU = mybir.AluOpType
AX = mybir.AxisListType
PM = mybir.MatmulPerfMode

B, L, CL, H, W = 8, 4, 256, 64, 64
C = L * CL          # 1024
HW = H * W          # 4096
P = C // 4          # 256
R = 2
EPS = 1e-5
NJ = C // 128       # 8   c-slabs
NCORES = 8

WSCALE = 16.0       # fp8 W1 and v0 pre-scale (exact: eps scaled to match)
DVE_COLS = 2368     # pass2 column split: DVE [0:2368], GpSimd [2368:4096]
ATTN_DMA_BC = False  # DMA can't read zero-stride partition APs; use PE

_CACHE = {}


def _build_nc():
    nc = bacc.Bacc()

    x_d = nc.dram_tensor("x", [C, HW], BF, kind="ExternalInput")
    wmc_d = nc.dram_tensor("wmc", [128, NJ], BF, kind="ExternalInput")
    cst_d = nc.dram_tensor("cst", [128, 128], F32, kind="ExternalInput")
    rhsi_d = nc.dram_tensor("rhsi", [128, 256], BF, kind="ExternalInput")
    sm_d = nc.dram_tensor("smalls", [128, 128], F32, kind="ExternalInput")
    w1_d = nc.dram_tensor("wg1", [128, 2, 16384], F8, kind="ExternalInput")
    w2_d = nc.dram_tensor("wg2", [128, 2, 4096], BF, kind="ExternalInput")
    out_d = nc.dram_tensor("out", [CL, HW], BF, kind="ExternalOutput")

    with tile.TileContext(nc) as tc, ExitStack() as ctx:
        resid = ctx.enter_context(tc.tile_pool(name="resid", bufs=1))
        spool = ctx.enter_context(tc.tile_pool(name="spool", bufs=1))
        scr = ctx.enter_context(tc.tile_pool(name="scr", bufs=2))
        dpool = ctx.enter_context(tc.tile_pool(name="diag", bufs=1))

        # ---- resident tiles ----
        x_sb = resid.tile([128, NJ, HW], BF, tag="x")
        wmc = resid.tile([128, NJ], BF, tag="wmc")
        cst = resid.tile([128, 128], F32, tag="cst")
        rhsi = resid.tile([128, 256], BF, tag="rhsi")
        sm = resid.tile([128, 128], F32, tag="sm")
        w1_sb = resid.tile([128, 2, 16384], F8, tag="w1")
        w2_sb = resid.tile([128, 2, 4096], BF, tag="w2")
        attn_bc = resid.tile([128, HW], BF, tag="attn_bc")
        stage = resid.tile([128, 2, HW], BF, tag="stage")

        # ---- DMA kickoff: x first, then consts, then gate weights ----
        nc.sync.dma_start(wmc[:], wmc_d[:])
        for j in range(NJ):
            nc.sync.dma_start(x_sb[:, j, :], x_d[128 * j:128 * (j + 1), :])
        nc.sync.dma_start(cst[:], cst_d[:])
        nc.sync.dma_start(sm[:], sm_d[:])
        nc.sync.dma_start(rhsi[:], rhsi_d[:])
        for s in range(2):
            nc.sync.dma_start(w1_sb[:, s], w1_d[:, s])
            nc.sync.dma_start(w2_sb[:, s], w2_d[:, s])

        ones_col = cst[:, 0:1]
        ones_row = cst[0:1, 0:128]
        ones_row_bf = rhsi[0:1, 128:256]

        dmy = spool.tile([1, 8], F32, tag="dmy", bufs=8)

        def act_warm(func):
            # dummy op: forces the act-table switch off the critical path
            nc.scalar.activation(dmy[0:1, 0:1], cst[0:1, 0:1], func)

        act_warm(AF.Exp)

        # ---- pass 1: logits row (wm stationary; rides the x DMA) ----
        attn_row = spool.tile([1, HW], BF, tag="attn_row")
        rowsum = spool.tile([1, 1], F32, tag="rowsum")
        with tc.tile_pool(name="psrow", bufs=1,
                          space=bass.MemorySpace.PSUM) as psrow:
            lg_row = psrow.tile([1, HW], F32, tag="row")
            for j in range(NJ):
                for nch in range(NJ):
                    nc.tensor.matmul(
                        lg_row[0:1, 512 * nch:512 * (nch + 1)],
                        wmc[:, j:j + 1],
                        x_sb[:, j, 512 * nch:512 * (nch + 1)],
                        start=(j == 0), stop=(j == NJ - 1),
                    )
            # exp + row-sum straight out of PSUM (|logits| < ~4: no max
            # subtraction needed; softmax is shift invariant)
            nc.scalar.activation(
                attn_row[:], lg_row[:], AF.Exp, accum_out=rowsum[:]
            )
        ps = ctx.enter_context(
            tc.tile_pool(name="ps", bufs=4, space=bass.MemorySpace.PSUM)
        )

        # inv16_bc = WSCALE / rowsum broadcast to all partitions
        inv = spool.tile([1, 1], F32, tag="inv")
        nc.vector.reciprocal(inv[:], rowsum[:])
        nc.vector.tensor_scalar_mul(inv[:], inv[:], WSCALE)
        ps_ib = ps.tile([128, 1], F32, tag="tiny", bufs=4)
        nc.tensor.matmul(ps_ib[:], ones_row, inv[:])
        inv16_bc = spool.tile([128, 1], F32, tag="inv16_bc")
        nc.vector.tensor_copy(inv16_bc[:], ps_ib[:])

        # ---- attn broadcast over partitions ----
        if ATTN_DMA_BC:
            nc.sync.dma_start(attn_bc[:], attn_row[0:1, :].partition_broadcast(128))
        else:
            for i in range(NJ):
                bc_ps = ps.tile([128, 512], F32, tag="big", name=f"bc_ps{i % 4}")
                nc.tensor.matmul(
                    bc_ps[:], ones_row_bf, attn_row[0:1, 512 * i:512 * (i + 1)]
                )
                if i % 2 == 0:
                    nc.vector.tensor_copy(attn_bc[:, 512 * i:512 * (i + 1)], bc_ps[:])
                else:
                    nc.scalar.copy(attn_bc[:, 512 * i:512 * (i + 1)], bc_ps[:])

        # ---- gate weight slicing helpers ----
        def w1_blk(s, t, m):
            off = (t * 16 + m) * 256
            return w1_sb[:, s, off:off + 256].rearrange("p (i q) -> p i q", i=2)

        def w2_blk(s, c2, kc):
            off = (c2 * 2 + kc) * 128
            return w2_sb[:, s, off:off + 128]

        # ---- pass 2: context. DVE runs 2x-mode bf16 multiplies for 6
        # slabs (Act reduces them via Identity+accum) and STT for the last
        # 2, roughly balancing both engines. The r0 W1 matmuls consume v0
        # pair-by-pair as it completes.
        v0 = spool.tile([128, NJ], F32, tag="v0")
        v0s = spool.tile([128, NJ], F32, tag="v0s")
        v0_f8 = spool.tile([128, NJ], F8, tag="v0f8")
        ps_h0 = ps.tile([128, 16], F32, tag="tiny", bufs=4)

        def stt_slab(j):
            sd = scr.tile([128, HW], BF, tag="sd")
            nc.vector.scalar_tensor_tensor(
                out=sd[:], in0=x_sb[:, j, :], scalar=1.0, in1=attn_bc[:],
                op0=ALU.mult, op1=ALU.mult, accum_out=v0[:, j:j + 1],
            )

        def tt_act_slab(j):
            sd = scr.tile([128, HW], BF, tag="sd")
            nc.vector.tensor_mul(sd[:], x_sb[:, j, :], attn_bc[:])
            sc = scr.tile([128, HW], BF, tag="sc")
            nc.scalar.activation(
                sc[:], sd[:], AF.Identity, accum_out=v0[:, j:j + 1]
            )

        PAIR_ORDER = [1, 2, 3, 0]  # STT slabs (pair 0) finish last on DVE

        def w1_pair(t):
            pr = slice(2 * t, 2 * t + 2)
            nc.vector.tensor_scalar_mul(v0s[:, pr], v0[:, pr], inv16_bc[:])
            nc.vector.tensor_copy(v0_f8[:, pr], v0s[:, pr])
            # r0 W1 (both branches stacked), fp8 DoubleRow: 2 k-slabs/matmul
            rhs = v0_f8[:, pr].rearrange("p (i n) -> p i n", i=2)
            for m in range(16):
                nc.tensor.matmul(
                    ps_h0[:, m:m + 1],
                    w1_blk(0, t, m),
                    rhs,
                    start=(t == PAIR_ORDER[0] and m == 0),
                    stop=(t == PAIR_ORDER[-1] and m == 15),
                    perf_mode=PM.DoubleRow,
                )

        for j in (2, 3, 4, 5, 6, 7):
            tt_act_slab(j)
            if j % 2 == 1:
                w1_pair(j // 2)
        stt_slab(0)
        stt_slab(1)
        w1_pair(0)

        # ---- per-branch LN + W2 tail (h half [128, 8] -> z half) ----
        def branch_tail(s, br, ps_hh, eps, name):
            o = 16 * s + 8 * br
            b1c = sm[:, o:o + 8]
            gc = sm[:, 32 + o:40 + o]
            bec = sm[:, 64 + o:72 + o]
            b2c = sm[:, 96 + o:104 + o]

            stats = spool.tile([128, 16], F32, tag="stats", bufs=4)
            nc.vector.tensor_add(stats[:, 0:8], ps_hh, b1c)
            nc.vector.tensor_mul(stats[:, 8:16], stats[:, 0:8], stats[:, 0:8])

            ps_st = ps.tile([1, 16], F32, tag="tiny", bufs=4)
            nc.tensor.matmul(ps_st[:], ones_col, stats[:])

            w8 = spool.tile([1, 12], F32, tag="w8", bufs=4)
            nc.vector.reduce_sum(
                out=w8[0:1, 0:4],
                in_=ps_st[0:1, 0:8].rearrange("p (s t) -> p s t", t=2),
                axis=AX.X,
            )
            nc.vector.reduce_sum(
                out=w8[0:1, 4:8],
                in_=ps_st[0:1, 8:16].rearrange("p (s t) -> p s t", t=2),
                axis=AX.X,
            )
            nc.vector.tensor_scalar_mul(w8[0:1, 0:4], w8[0:1, 0:4], 1.0 / P)
            nc.vector.tensor_scalar_mul(w8[0:1, 4:8], w8[0:1, 4:8], 1.0 / P)
            nc.vector.tensor_mul(w8[0:1, 8:12], w8[0:1, 0:4], w8[0:1, 0:4])
            nc.vector.tensor_sub(w8[0:1, 4:8], w8[0:1, 4:8], w8[0:1, 8:12])
            # 1/sigma = sqrt(1/(var + eps))
            nc.vector.tensor_scalar_add(w8[0:1, 4:8], w8[0:1, 4:8], eps)
            nc.vector.reciprocal(w8[0:1, 4:8], w8[0:1, 4:8])
            nc.scalar.activation(w8[0:1, 4:8], w8[0:1, 4:8], AF.Sqrt)

            brow = spool.tile([1, 16], F32, tag="brow", bufs=4)
            bview = brow[0:1, 0:8].rearrange("p (s t) -> p t s", t=2)
            iview = brow[0:1, 8:16].rearrange("p (s t) -> p t s", t=2)
            for t in range(2):
                nc.vector.tensor_copy(bview[:, t, :], w8[0:1, 0:4])
                nc.vector.tensor_copy(iview[:, t, :], w8[0:1, 4:8])

            ps_bc = ps.tile([128, 16], F32, tag="tiny", bufs=4)
            nc.tensor.matmul(ps_bc[:], ones_row, brow[:])
            bc = spool.tile([128, 16], F32, tag="bc", bufs=4)
            nc.vector.tensor_copy(bc[:], ps_bc[:])

            hn = spool.tile([128, 8], F32, tag="hn", bufs=4)
            nc.vector.tensor_sub(hn[:], stats[:, 0:8], bc[:, 0:8])
            nc.vector.tensor_mul(hn[:], hn[:], bc[:, 8:16])
            nc.vector.tensor_mul(hn[:], hn[:], gc)
            nc.vector.tensor_add(hn[:], hn[:], bec)
            h8 = spool.tile([128, 8], BF, tag="h8", bufs=4)
            nc.vector.tensor_scalar_max(h8[:], hn[:], 0.0)  # relu + cast

            ps_z = ps.tile([128, 8], F32, tag="tiny", bufs=4)
            for cc in range(8):
                base = (cc // 2) * 2
                for kc in range(2):
                    nc.tensor.matmul(
                        ps_z[:, cc:cc + 1],
                        w2_blk(s, 8 * br + cc, kc),
                        h8[:, base + kc:base + kc + 1],
                        start=(cc == 0 and kc == 0),
                        stop=(cc == 7 and kc == 1),
                    )
            zb = spool.tile([128, 8], F32, tag=name)
            nc.vector.tensor_add(zb[:], ps_z[:], b2c)
            return zb

        def sigmoid8(dst, src):
            # sigmoid(z) = 0.5 + 0.5*tanh(z/2); tanh shares the exp table set
            t8 = spool.tile([128, 8], F32, tag="t8", bufs=2)
            nc.scalar.activation(t8[:], src, AF.Tanh, scale=0.5)
            nc.vector.tensor_scalar_mul(dst, t8[:], 0.5)
            nc.vector.tensor_scalar_add(dst, dst, 0.5)

        # r0 mul tail first: it gates the r1 mul chain -> pass3
        zb0m = branch_tail(0, 0, ps_h0[:, 0:8], EPS * 65536.0, "zb0m")
        vmul = spool.tile([128, NJ], F32, tag="vmul")
        sigmoid8(vmul[:], zb0m[:])
        v1m_f8 = spool.tile([128, NJ], F8, tag="v1mf8")
        nc.vector.tensor_copy(v1m_f8[:], vmul[:])

        zb0a = branch_tail(0, 1, ps_h0[:, 8:16], EPS * 65536.0, "zb0a")
        v1a_f8 = spool.tile([128, NJ], F8, tag="v1af8")
        nc.vector.tensor_copy(v1a_f8[:], zb0a[:])

        # ---- r1 W1: mul branch fully first (m-major), then add half 1 ----
        ps_h1m = ps.tile([128, 8], F32, tag="tiny", bufs=4)
        for m in range(8):
            for t in range(4):
                nc.tensor.matmul(
                    ps_h1m[:, m:m + 1],
                    w1_blk(1, t, m),
                    v1m_f8[:, 2 * t:2 * t + 2].rearrange("p (i n) -> p i n", i=2),
                    start=(m == 0 and t == 0),
                    stop=(m == 7 and t == 3),
                    perf_mode=PM.DoubleRow,
                )
        ps_h1a = ps.tile([128, 8], F32, tag="tiny", bufs=4)

        def w1_r1_add(ms):
            for m in ms:
                for t in range(4):
                    nc.tensor.matmul(
                        ps_h1a[:, m - 8:m - 7],
                        w1_blk(1, t, m),
                        v1a_f8[:, 2 * t:2 * t + 2].rearrange(
                            "p (i n) -> p i n", i=2),
                        start=(m == 8 and t == 0),
                        stop=(m == 15 and t == 3),
                        perf_mode=PM.DoubleRow,
                    )

        w1_r1_add(range(8, 12))  # half 1 fills PE while LN1m runs

        zb1m = branch_tail(1, 0, ps_h1m[:], EPS * 256.0, "zb1m")
        mm_f = spool.tile([128, NJ], F32, tag="mmf")
        sigmoid8(mm_f[:], zb1m[:])
        nc.vector.tensor_add(mm_f[:], mm_f[:], vmul[:])

        diags = []
        for js in range(NJ):
            dt_ = dpool.tile([128, 128], BF, tag=f"diag{js}", name=f"diag{js}")
            nc.vector.tensor_scalar_mul(dt_[:], rhsi[:, 0:128], mm_f[:, js:js + 1])
            diags.append(dt_)

        w1_r1_add(range(12, 16))  # half 2

        addsum = spool.tile([128, 2], F32, tag="addsum")

        def p3_mm(jj, nch):
            ps_o = ps.tile([128, 512], F32, tag="big")
            for lv in range(4):
                js = 2 * lv + jj
                nc.tensor.matmul(
                    ps_o[:],
                    diags[js][:],
                    x_sb[:, js, 512 * nch:512 * (nch + 1)],
                    start=(lv == 0), stop=(lv == 3),
                )
            return ps_o

        def p3_evac(jj, nch, ps_o):
            stg = stage[:, jj, 512 * nch:512 * (nch + 1)]
            if nch % 2 == 0:
                nc.scalar.activation(
                    stg, ps_o[:], AF.Identity,
                    bias=addsum[:, jj:jj + 1], scale=1.0,
                )
            else:
                nc.vector.tensor_scalar_add(stg, ps_o[:], addsum[:, jj:jj + 1])

        def p3_chunk(jj, nch):
            p3_evac(jj, nch, p3_mm(jj, nch))

        # first pass3 chunks fill PE while the r1 add-branch tail runs;
        # their evacs are deferred until addsum exists (program-order deps)
        head = [p3_mm(0, nch) for nch in range(4)]

        zb1a = branch_tail(1, 1, ps_h1a[:], EPS * 256.0, "zb1a")
        ma_f = spool.tile([128, NJ], F32, tag="maf")
        nc.vector.tensor_add(ma_f[:], zb1a[:], zb0a[:])
        nc.vector.reduce_sum(
            out=addsum[:],
            in_=ma_f[:].rearrange("p (l t) -> p t l", t=2),
            axis=AX.X,
        )

        for nch in range(4):
            p3_evac(0, nch, head[nch])
        for nch in range(4, NJ):
            p3_chunk(0, nch)
        for g in range(4):
            nc.sync.dma_start(
                out_d[0:128, 1024 * g:1024 * (g + 1)],
                stage[:, 0, 1024 * g:1024 * (g + 1)],
            )
        for nch in range(NJ):
            p3_chunk(1, nch)
        for g in range(4):
            nc.sync.dma_start(
                out_d[128:256, 1024 * g:1024 * (g + 1)],
                stage[:, 1, 1024 * g:1024 * (g + 1)],
            )

    nc.compile()
    return nc


def _pack_inputs(x0, x1, x2, x3, wm, bm,
                 add_W1, add_b1, add_g, add_be, add_W2, add_b2,
                 mul_W1, mul_b1, mul_g, mul_be, mul_W2, mul_b2):
    bf = ml_dtypes.bfloat16
    f8 = ml_dtypes.float8_e4m3
    f32 = np.float32

    wmc = np.asarray(wm, f32).reshape(NJ, 128).T.astype(bf).copy()
    cst = np.ones((128, 128), f32)
    rhsi = np.ones((128, 256), bf)
    rhsi[:, 0:128] = np.eye(128, dtype=bf)

    w1h = np.zeros((128, 2, 16384), f8)
    w2h = np.zeros((128, 2, 4096), bf)
    sm = np.zeros((128, 128), f32)
    for s in range(2):
        w1s = np.concatenate([
            np.asarray(mul_W1[s], f32).reshape(C, C),
            np.asarray(add_W1[s], f32).reshape(C, C),
        ], axis=0) * WSCALE                                # [2048, 1024]
        # w1h[pp, s, (t,m,i,q)] = w1s[128m+q, 256t+128i+pp]
        arr = w1s.reshape(16, 128, 4, 2, 128)              # [m, q, t, i, pp]
        w1h[:, s, :] = arr.transpose(4, 2, 0, 3, 1).reshape(128, 16384).astype(f8)

        w2s = np.stack([
            np.asarray(mul_W2[s], f32),                     # [L, CL, P]
            np.asarray(add_W2[s], f32),
        ])                                                  # [2, L, CL, P]
        # w2h[pin, s, ((8b+2l+m2)*2+kc)*128+q] = w2s[b, l, 128m2+q, 128kc+pin]
        arr2 = w2s.reshape(2, 4, 2, 128, 2, 128)            # [b, l, m2, q, kc, pin]
        w2h[:, s, :] = arr2.transpose(5, 0, 1, 2, 4, 3).reshape(128, 4096).astype(bf)

        def colpack(mu, ad):
            v = np.concatenate([np.asarray(mu, f32).reshape(C),
                                np.asarray(ad, f32).reshape(C)])
            return v.reshape(16, 128).T

        sm[:, 16 * s:16 * s + 16] = colpack(mul_b1[s], add_b1[s])
        sm[:, 32 + 16 * s:48 + 16 * s] = colpack(mul_g[s], add_g[s])
        sm[:, 64 + 16 * s:80 + 16 * s] = colpack(mul_be[s], add_be[s])
        b2s = np.stack([np.asarray(mul_b2[s], f32),
                        np.asarray(add_b2[s], f32)])        # [2, L, CL]
        sm[:, 96 + 16 * s:112 + 16 * s] = (
            b2s.reshape(2, 4, 2, 128).transpose(3, 0, 1, 2).reshape(128, 16)
        )

    shared = dict(wmc=wmc, cst=cst, rhsi=rhsi, smalls=sm, wg1=w1h, wg2=w2h)

    in_maps = []
    xs = [np.asarray(a, f32) for a in (x0, x1, x2, x3)]
    for b in range(B):
        xc = np.concatenate(
            [a[b].reshape(CL, HW) for a in xs], axis=0
        ).astype(bf)
        in_maps.append({"x": xc, **shared})
    return in_maps


def kernel(**inputs):
    from concourse.bass_utils import run_bass_kernel_spmd

    if "nc" not in _CACHE:
        _CACHE["nc"] = _build_nc()
    nc = _CACHE["nc"]

    in_maps = _pack_inputs(**inputs)
    res = run_bass_kernel_spmd(nc, in_maps, list(range(NCORES)))
    _CACHE["last_results"] = res
    out = np.stack(
        [res.results[b]["out"].reshape(CL, H, W) for b in range(B)]
    ).astype(np.float32)
    return out
